# revision 34
# baseline (speedup 1.0000x reference)
"""BiModalAttention Trainium2 kernel (v3).

Full-input contract: kernel(mode1, mode2) -> [S, B, 2D] float32.
mode1/mode2: [S=1024, B=32, D=1024] float32.

Reference computation per batch b (m1 = mode1[:, b, :], m2 = mode2[:, b, :]):
    C1 = m1 @ m2.T                  # [S, S]
    a1 = softmax_rows(C1) @ m2 * m1
    a2 = softmax_rows(C1.T) @ m1 * m2
    out[:, b, :] = concat([a1, a2], -1)

Sharding: batch dim across 8 NeuronCores (4 batch elements per core).

v3 structure (per batch element) — changes vs v2:
  A. C1 in fp32r as before (bf16 scores measured 5.8e-2 rel err: argmax
     flips; fp32r issues at ~227ns/MM = near the bf16 N=512 roofline anyway).
     Evacuation stays ScalarE copy + DVE row-max as in v2 — a fused
     tensor_tensor_reduce evacuation (PSUM in0 + AP-init chained accum)
     passed CoreSim but hard-wedged the exec unit at NEFF load
     (NRT_EXEC_UNIT_UNRECOVERABLE); do not resurrect it.
  B. E1 = exp(C1 - rm1[s]) directly from the fp32 strip in ONE ACT op
     (per-partition bias = -rm1, accum_out = Z1). The old rm1-broadcast
     (DVE bcast + PE transpose + evac) and the separate Z1 pass are gone.
  C. rm2 from transposing a bf16 COPY of C1 (ACT cast): any softmax shift
     within +-80 of the true column max works as long as it is used
     consistently, so bf16 (+-1) precision is plenty. bf16 transposes run
     1 pass (~109ns) vs fp32 LOW_HIGH 2 passes, and the [128,1024] bf16
     transpose group fits ONE PSUM bank; rm2 is reduced straight out of
     PSUM (negated) — the transposed C2 is never materialized in SBUF.
  D. E1T for the o1 matmul by transposing the bf16 E1 strips (8 groups,
     DVE-evacuated). v2 instead transposed fp32 C1 and re-did the shift+exp
     on the transposed side (DVE add + ACT exp per strip) — all gone.
  E. E2T = exp(C1 - rm2[t]) as in v2 (DVE add of the rm2 row-broadcast +
     ACT exp). Z2 now comes from tiny N=1 ones-column matmuls accumulated
     in PSUM during the AV2 k-loop (weights already loaded there), replacing
     v2's extra ACT exp-accumulate passes over transposed strips. Z2 summing
     the SAME bf16 E2T values as the numerator also cancels the bf16
     rounding in the softmax ratio (v3 measures ~3.9e-3 vs v2's 4.3e-3).
  F. AV matmuls in bf16 exactly as v2: out = (psum * (1/Z)[part]) * gate.
     The r1/r2 rhs/gate loads are issued at the START of P2 so they run
     during the transpose phase and sit AHEAD of batch j+1's 8MB fp32
     score-input loads in the DMA queue (they used to stall AV by ~3-4us
     per chunk). First batch's score inputs load in k-pair quarters to cut
     the cold-start PE stall from ~27us to ~8us.
  Keepers (small bf16 matmuls) follow every transpose group:
  transpose-mode does not count as PE-activity for the HAM clock gate, so
  a transpose-only stretch re-throttles the PE to 1.2 GHz without them.
  Emission order P2(j) -> P3(j) -> P1(j+1): with scores between softmax
  and AV (the v2 order), batch j+1's rm1 DVE reductions queued ahead of
  batch j's AV-evacuation stt ops on the FIFO DVE queue and stalled every
  AV start by 2-4us. Scores inputs prefetch during P2(j) behind the r
  loads. Z2 ones-matmuls run in their own k-loop after each AV2 group so
  they don't break AV matmul pipelining.
  Measured: v2 569.6us -> v3 500.3us (rel err 4.32e-3, gate 2e-2).
"""

import os
os.environ.setdefault("NEURON_RT_RESET_CORES", "1")
import time

import numpy as np

import concourse.bacc as bacc
import concourse.mybir as mybir
import concourse.tile as tile
from concourse.masks import make_identity
from concourse.bass_utils import run_bass_kernel_spmd

S = 1024
D = 1024
B = 32
N_CORES = 8
BPC = B // N_CORES          # batch elements per core
P = 128                     # partitions
NK = S // P                 # contraction tiles (8)
NI = S // P                 # s tiles (8)
CW = 512                    # AV d-chunk width (bf16 matmul moving dim)
NCH = D // CW               # AV chunks (2)

f32 = mybir.dt.float32
f32r = mybir.dt.float32r
bf16 = mybir.dt.bfloat16
AX = mybir.AxisListType
ALU = mybir.AluOpType
ACTF = mybir.ActivationFunctionType


def _load_p1_inputs(nc, sb, st, j, m1t, m2t):
    # chunked loads, m1t/m2t interleaved, so the C1 k-loop can start on the
    # first k-pair while the rest is still in flight (first batch: quarters
    # to cut the cold-start stall; later batches prefetch under P2/P3)
    m1t_sb = st["m1t_sb"] = sb.tile([P, NK, S], f32r, tag="m1t", bufs=1,
                                    name=f"m1t_sb{j}")
    m2t_sb = st["m2t_sb"] = sb.tile([P, NK, S], f32r, tag="m2t", bufs=1,
                                    name=f"m2t_sb{j}")
    bounds = (0, 2, 4, 6, NK) if j == 0 else (0, NK // 2, NK)
    for (lo, hi) in zip(bounds, bounds[1:]):
        nc.gpsimd.dma_start(
            out=m1t_sb[:, lo:hi, :],
            in_=m1t[j].rearrange("(k p) s -> p k s", p=P)[:, lo:hi, :])
        nc.gpsimd.dma_start(
            out=m2t_sb[:, lo:hi, :],
            in_=m2t[j].rearrange("(k p) s -> p k s", p=P)[:, lo:hi, :])


def _emit_p1(nc, sb, ps, zcol, st, j, m1n, m2n):
    # ---- Phase 1: C1 scores (fp32r) + E1/Z1/rm1 + bf16 C1 copy ----
    m1t_sb, m2t_sb = st["m1t_sb"], st["m2t_sb"]

    # AV rhs/gate loads issued a full phase ahead of use: they stream during
    # the scores+transpose phases, so P3 never waits on them (issuing them
    # at P2 start left ~3.7us PE gaps at each AV start)
    st["r"] = []
    for c in range(NCH):
        c0 = c * CW
        r2 = sb.tile([P, NK, CW], bf16, tag="rhs", bufs=4, name=f"r2_{j}_{c}")
        r1 = sb.tile([P, NK, CW], bf16, tag="rhs", bufs=4, name=f"r1_{j}_{c}")
        nc.gpsimd.dma_start(
            out=r2, in_=m2n[j].rearrange("(k p) d -> p k d", p=P)[:, :, c0:c0 + CW])
        nc.gpsimd.dma_start(
            out=r1, in_=m1n[j].rearrange("(k p) d -> p k d", p=P)[:, :, c0:c0 + CW])
        st["r"].append((r2, r1))

    c1 = st["c1"] = []
    c1b = st["c1b"] = []
    e1 = st["e1"] = []
    rm1 = sb.tile([P, NI], f32, tag="rm1", bufs=2, name=f"rm1_{j}")
    z1 = st["z1"] = sb.tile([P, NI], f32, tag="z1", bufs=2, name=f"z1_{j}")
    for i in range(NI):
        c1_i = sb.tile([P, S], f32, tag="c1", bufs=NI, name=f"c1_{j}_{i}")
        c1.append(c1_i)
        for n in range(2):
            if j == 0:
                # batch 0 is DMA-paced: split the k-accumulation into two
                # PSUM groups so the first half runs while the second half
                # of the score inputs is still in flight, then combine on
                # evacuation (single-PSUM-operand add)
                pca = ps.tile([P, 512], f32, tag="c", bufs=3, name=f"pca{j}_{i}_{n}")
                for k in range(NK // 2):
                    nc.tensor.matmul(
                        pca,
                        m1t_sb[:, k, i * P:(i + 1) * P],
                        m2t_sb[:, k, n * 512:(n + 1) * 512],
                        start=(k == 0),
                        stop=(k == NK // 2 - 1),
                    )
                pcb = ps.tile([P, 512], f32, tag="c", bufs=3, name=f"pcb{j}_{i}_{n}")
                for k in range(NK // 2, NK):
                    nc.tensor.matmul(
                        pcb,
                        m1t_sb[:, k, i * P:(i + 1) * P],
                        m2t_sb[:, k, n * 512:(n + 1) * 512],
                        start=(k == NK // 2),
                        stop=(k == NK - 1),
                    )
                nc.scalar.copy(out=c1_i[:, n * 512:(n + 1) * 512], in_=pcb)
                nc.vector.tensor_add(c1_i[:, n * 512:(n + 1) * 512], pca,
                                     c1_i[:, n * 512:(n + 1) * 512])
            else:
                pc = ps.tile([P, 512], f32, tag="c", bufs=3, name=f"pc{j}_{i}_{n}")
                for k in range(NK):
                    nc.tensor.matmul(
                        pc,
                        m1t_sb[:, k, i * P:(i + 1) * P],
                        m2t_sb[:, k, n * 512:(n + 1) * 512],
                        start=(k == 0),
                        stop=(k == NK - 1),
                    )
                nc.scalar.copy(out=c1_i[:, n * 512:(n + 1) * 512], in_=pc)
        nc.vector.tensor_reduce(rm1[:, i:i + 1], c1_i, axis=AX.X,
                                op=ALU.max, negate=True)
        e1_i = sb.tile([P, S], bf16, tag="e1", bufs=NI, name=f"e1_{j}_{i}")
        e1.append(e1_i)
        nc.scalar.activation(e1_i, c1_i, ACTF.Exp, bias=rm1[:, i:i + 1],
                             accum_out=z1[:, i:i + 1])
        c1b_i = sb.tile([P, S], bf16, tag="c1b", bufs=NI, name=f"c1b_{j}_{i}")
        c1b.append(c1b_i)
        nc.scalar.activation(c1b_i, c1_i, ACTF.Copy)


def _keeper(nc, ps, kc, nm):
    # tiny discarded bf16 matmul: transpose-mode doesn't count as PE activity
    # for the HAM clock gate, so a transpose-only stretch would re-throttle
    # the PE to 1.2 GHz without these
    pk = ps.tile([P, 512], f32, tag="av", bufs=2, name=nm)
    nc.tensor.matmul(pk, kc[:, 0:P], kc, start=True, stop=True)


def _emit_p2(nc, sb, ps, identb, kc, st, j):
    c1, c1b, e1 = st["c1"], st["c1b"], st["e1"]

    invz1 = st["invz1"] = sb.tile([P, NI], f32, tag="invz1", bufs=2, name=f"invz1_{j}")
    nc.vector.reciprocal(invz1, st["z1"])

    # ---- rm2 via bf16 transposes of C1, reduced straight out of PSUM ----
    rm2p = sb.tile([P, NK], f32, tag="rm2p", bufs=2, name=f"rm2p_{j}")
    for t in range(NK):
        ptb = ps.tile([P, S], bf16, tag="pt", bufs=2, name=f"ptb_{j}_{t}")
        for i in range(NI):
            nc.tensor.transpose(ptb[:, i * P:(i + 1) * P],
                                c1b[i][:, t * P:(t + 1) * P], identb)
        nc.vector.tensor_reduce(rm2p[:, t:t + 1], ptb, axis=AX.X,
                                op=ALU.max, negate=True)
        _keeper(nc, ps, kc, f"kp1_{j}_{t}")

    # ---- E1T strips by transposing bf16 E1 ----
    e1t = st["e1t"] = []
    for t in range(NK):
        pte = ps.tile([P, S], bf16, tag="pt", bufs=2, name=f"pte_{j}_{t}")
        for i in range(NI):
            nc.tensor.transpose(pte[:, i * P:(i + 1) * P],
                                e1[i][:, t * P:(t + 1) * P], identb)
        e1t_t = sb.tile([P, S], bf16, tag="e1t", bufs=NK, name=f"e1t_{j}_{t}")
        e1t.append(e1t_t)
        nc.vector.tensor_copy(e1t_t, pte)
        _keeper(nc, ps, kc, f"kp2_{j}_{t}")

    # ---- rm2 broadcast across partitions (rm2b[s, t] = -rm2[t]) ----
    rm2b = sb.tile([P, S], f32, tag="rm2b", bufs=1, name=f"rm2b_{j}")
    ptr = ps.tile([P, S], bf16, tag="pt", bufs=2, name=f"ptr_{j}")
    for t in range(NK):
        xb = sb.tile([P, P], bf16, tag="xb", bufs=2, name=f"xb_{j}_{t}")
        nc.vector.tensor_copy(xb, rm2p[:, t:t + 1].broadcast_to([P, P]))
        nc.tensor.transpose(ptr[:, t * P:(t + 1) * P], xb, identb)
    nc.vector.tensor_copy(rm2b, ptr)

    # ---- E2T = exp(C1 - rm2[t]) ----
    e2 = st["e2"] = []
    for i in range(NI):
        e2_i = sb.tile([P, S], bf16, tag="e2", bufs=NI, name=f"e2_{j}_{i}")
        e2.append(e2_i)
        # in-place shift: the fp32 C1 strip is dead after this
        nc.vector.tensor_add(c1[i], c1[i], rm2b)
        nc.scalar.activation(e2_i, c1[i], ACTF.Exp)


def _emit_p3(nc, sb, ps, st, j, ones, outp):
    e1t, e2 = st["e1t"], st["e2"]
    invz1 = st["invz1"]
    pz = ps.tile([P, NI], f32, tag="pz", bufs=1, name=f"pz_{j}")
    invz2 = sb.tile([P, NI], f32, tag="invz2", bufs=2, name=f"invz2_{j}")
    for c in range(NCH):
        c0 = c * CW
        r2, r1 = st["r"][c]

        for i in range(NI):
            for (es, rhs, gate, dbase) in (
                (e1t, r2, r1, 0),
                (e2, r1, r2, D),
            ):
                pav = ps.tile([P, CW], f32, tag="av", bufs=2,
                              name=f"pav{j}_{c}_{i}_{dbase}")
                dir2 = dbase != 0
                for k in range(NK):
                    nc.tensor.matmul(
                        pav,
                        es[k][:, i * P:(i + 1) * P],
                        rhs[:, k, :],
                        start=(k == 0),
                        stop=(k == NK - 1),
                    )
                if dir2 and c == 0:
                    # Z2[t]: ones-column matmuls in their own k-loop so they
                    # don't break the AV2 matmul pipelining
                    for k in range(NK):
                        nc.tensor.matmul(
                            pz[:, i:i + 1],
                            es[k][:, i * P:(i + 1) * P],
                            ones,
                            start=(k == 0),
                            stop=(k == NK - 1),
                        )
                    nc.vector.reciprocal(invz2[:, i:i + 1], pz[:, i:i + 1])
                invz = invz2 if dir2 else invz1
                a_sb = sb.tile([P, CW], f32, tag="ao", bufs=3,
                               name=f"a{j}_{c}_{i}_{dbase}")
                nc.vector.scalar_tensor_tensor(
                    a_sb, pav, invz[:, i:i + 1],
                    gate[:, i, :],
                    op0=ALU.mult, op1=ALU.mult)
                nc.sync.dma_start(
                    out=outp[j, i * P:(i + 1) * P,
                             dbase + c0:dbase + c0 + CW],
                    in_=a_sb)


def _build():
    nc = bacc.Bacc("TRN2", target_bir_lowering=False, debug=False,
                   num_devices=N_CORES)
    m1n = nc.dram_tensor("m1n", [BPC, S, D], f32, kind="ExternalInput").ap()
    m2n = nc.dram_tensor("m2n", [BPC, S, D], f32, kind="ExternalInput").ap()
    m1t = nc.dram_tensor("m1t", [BPC, D, S], f32, kind="ExternalInput").ap()
    m2t = nc.dram_tensor("m2t", [BPC, D, S], f32, kind="ExternalInput").ap()
    outp = nc.dram_tensor("out", [BPC, S, 2 * D], f32, kind="ExternalOutput").ap()

    with tile.TileContext(nc) as tc:
        with tc.tile_pool(name="consts", bufs=1) as consts, \
             tc.tile_pool(name="sb", bufs=1) as sb, \
             tc.tile_pool(name="ps", bufs=1, space="PSUM") as ps:
            identb = consts.tile([P, P], bf16)
            make_identity(nc, identb)
            kc = consts.tile([P, 512], bf16)
            nc.vector.memset(kc, 1.0)
            ones = consts.tile([P, 1], bf16)
            nc.vector.memset(ones, 1.0)
            zcol = consts.tile([P, 1], f32)
            nc.vector.memset(zcol, 0.0)
            # Software-pipelined emission: P1(j+1) sits AFTER P3(j). With
            # P1(j+1) between P2(j) and P3(j) (the v2 order), batch j+1's
            # rm1 DVE reductions queued ahead of batch j's AV evacuation
            # stt ops on the FIFO DVE queue and stalled the AV phase 2-4us
            # per start. Scores inputs for j+1 prefetch during P2(j) (behind
            # the r loads on the DMA queue) so P1(j+1) never waits on DMA.
            # P1(j+1) must still come after P2(j): the c1 strip slots are
            # freed by P2(j)'s e2 exps (head-of-line deadlock otherwise).
            sts = [dict() for _ in range(BPC)]
            _load_p1_inputs(nc, sb, sts[0], 0, m1t, m2t)
            _emit_p1(nc, sb, ps, zcol, sts[0], 0, m1n, m2n)
            for j in range(BPC):
                _emit_p2(nc, sb, ps, identb, kc, sts[j], j)
                if j + 1 < BPC:
                    _load_p1_inputs(nc, sb, sts[j + 1], j + 1, m1t, m2t)
                _emit_p3(nc, sb, ps, sts[j], j, ones, outp)
                if j + 1 < BPC:
                    _emit_p1(nc, sb, ps, zcol, sts[j + 1], j + 1, m1n, m2n)
    nc.compile()
    return nc


_NC_CACHE = None


def _get_nc():
    global _NC_CACHE
    if _NC_CACHE is None:
        _NC_CACHE = _build()
    return _NC_CACHE


def kernel(mode1: np.ndarray, mode2: np.ndarray, _trace: bool = False,
           _result_box: dict | None = None) -> np.ndarray:
    mode1 = np.asarray(mode1, dtype=np.float32)
    mode2 = np.asarray(mode2, dtype=np.float32)

    m1n_all = np.ascontiguousarray(mode1.transpose(1, 0, 2))  # [B, S, D]
    m2n_all = np.ascontiguousarray(mode2.transpose(1, 0, 2))
    m1t_all = np.ascontiguousarray(mode1.transpose(1, 2, 0))  # [B, D, S]
    m2t_all = np.ascontiguousarray(mode2.transpose(1, 2, 0))

    nc = _get_nc()
    in_maps = []
    for c in range(N_CORES):
        lo, hi = c * BPC, (c + 1) * BPC
        in_maps.append({
            "m1n": m1n_all[lo:hi],
            "m2n": m2n_all[lo:hi],
            "m1t": m1t_all[lo:hi],
            "m2t": m2t_all[lo:hi],
        })

    r = None
    last_err = None
    for attempt in range(3):
        try:
            r = run_bass_kernel_spmd(nc, in_maps, list(range(N_CORES)),
                                     trace=_trace)
            break
        except Exception as e:  # transient NRT exec-unit errors recover on retry
            last_err = e
            time.sleep(2.0)
    if r is None:
        raise last_err
    if _result_box is not None:
        _result_box["result"] = r

    out = np.empty((S, B, 2 * D), dtype=np.float32)
    for c in range(N_CORES):
        res = r.results[c]["out"]  # [BPC, S, 2D]
        out[:, c * BPC:(c + 1) * BPC, :] = res.transpose(1, 0, 2)
    return out
